# revision 1
# baseline (speedup 1.0000x reference)
"""Trainium2 8-core kernel for the paired contrastive (NT-Xent-like) loss.

Math (tau=0.5, N=8192, D=256):
    z1 = l2norm(H_1), z2 = l2norm(H_2)
    den1_i = sum_j exp(z1.z1/t) + sum_j exp(z1.z2/t) - exp(|z1_i|^2/t)
    den2_i = sum_j exp(z2.z2/t) + sum_j exp(z2.z1/t) - exp(|z2_i|^2/t)
    loss = (1/2N) * sum_i [ ln(den1_i) + ln(den2_i) - 2*(z1_i.z2_i)/t ]
with |z_i|^2 == 1 analytically, so the subtracted diagonal is exp(1/t) = e^2.

Sharding: rows split across 8 cores (1024 each); every core holds the full
(transposed, bf16) embeddings as the moving matmul operand.  Three exp
streams per core (S12, S11, S22); the S21 row-sums come from column-sums of
exp(S12/t) via a cross-core ReduceScatter (S21 = S12^T).  Each core emits one
partial scalar; the host sums them and divides by 2N.
"""

import math

import numpy as np
import ml_dtypes

import concourse.bass as bass
import concourse.bass_isa as bass_isa
import concourse.tile as tile
from concourse import bacc, mybir
from concourse.bass_utils import run_bass_kernel_spmd

F32 = mybir.dt.float32
BF16 = mybir.dt.bfloat16
AF = mybir.ActivationFunctionType
ALU = mybir.AluOpType
AX = mybir.AxisListType

TAU = 0.5
E2 = math.exp(1.0 / TAU)  # analytic diag of the "refl" exp-similarity

N_FULL, D_FULL, N_CORES = 8192, 256, 8


def build_nc(N=N_FULL, D=D_FULL, n_cores=N_CORES):
    """Build the SPMD graph for one core (same graph runs on all cores)."""
    R = N // n_cores           # rows owned per core
    NK = D // 128              # contraction k-tiles
    CH = 512                   # column chunk (one PSUM bank of f32)
    NCH = N // CH              # chunks across full column dim
    CHB = min(CH, R)           # block column chunk width
    NCHB = (R + CH - 1) // CH  # chunks across block rows
    G = min(2048, N)           # exp-group width (4 PSUM banks)
    NG = N // G
    GCH = G // CH
    NRT = R // 128             # 128-row tiles per core
    L = 2 * N + 2 * R          # flat norm-vector length
    SS = 2 * NCH + 2 * NCHB    # stacked norm rows

    assert R % 128 == 0 and D % 128 == 0 and N % CH == 0 and N % G == 0
    assert SS <= 128

    nc = bacc.Bacc("TRN2", target_bir_lowering=False, debug=False,
                   num_devices=n_cores)

    ht = [nc.dram_tensor("HT1", [D, N], BF16, kind="ExternalInput"),
          nc.dram_tensor("HT2", [D, N], BF16, kind="ExternalInput")]
    hb = [nc.dram_tensor("Hb1", [D, R], BF16, kind="ExternalInput"),
          nc.dram_tensor("Hb2", [D, R], BF16, kind="ExternalInput")]
    out = nc.dram_tensor("out", [1, 1], F32, kind="ExternalOutput")

    with tile.TileContext(nc) as tc, \
         tc.tile_pool(name="persist", bufs=1) as per, \
         tc.tile_pool(name="dram", bufs=1, space="DRAM") as dram:
        # --- persistent tensors ---------------------------------------
        Z = [[per.tile([128, N], BF16, tag=f"z{t}{k}", name=f"z{t}{k}")
              for k in range(NK)] for t in range(2)]
        Zb = [[per.tile([128, R], BF16, tag=f"zb{t}{k}", name=f"zb{t}{k}")
               for k in range(NK)] for t in range(2)]
        rows = {st: per.tile([128, NRT], F32, tag=f"rows_{st}",
                             name=f"rows_{st}")
                for st in ("s11", "s12", "s22")}
        colacc = per.tile([128, N], F32, tag="colacc", name="colacc")
        dn = per.tile([128, NRT], F32, tag="dn", name="dn")
        sska = per.tile([NCH + 2 * NCHB, CH], F32, tag="sska", name="sska")
        sskb = per.tile([NCH, CH], F32, tag="sskb", name="sskb")
        rvka = per.tile([NCH + 2 * NCHB, CH], BF16, tag="rvka", name="rvka")
        rvkb = per.tile([NCH, CH], BF16, tag="rvkb", name="rvkb")
        ii_tot = per.tile([1, 1], F32, tag="ii_tot", name="ii_tot")
        lnacc = per.tile([128, 1], F32, tag="lnacc", name="lnacc")
        ones_k = per.tile([128, 1], BF16, tag="ones_k", name="ones_k")
        ones_b = per.tile([1, 128], BF16, tag="ones_b", name="ones_b")
        ones_f = per.tile([128, 1], F32, tag="ones_f", name="ones_f")
        zb = per.tile([128, 1], F32, tag="zb", name="zb")
        cc_in = dram.tile([N], F32, tag="cc_in", name="cc_in")
        cc_out = dram.tile([R], F32, tag="cc_out", name="cc_out")

        nc.gpsimd.memset(ones_k[:], 1.0)
        nc.gpsimd.memset(ones_b[:], 1.0)
        nc.gpsimd.memset(ones_f[:], 1.0)
        nc.gpsimd.memset(zb[:], 0.0)
        nc.gpsimd.memset(sska[:], 1.0)  # unused lanes stay recip/sqrt-legal
        nc.gpsimd.memset(sskb[:], 1.0)

        # Two independent norm chains: pass A (Z1 + blocks) feeds streams
        # s12/s22 early; pass B (Z0, only needed by the last stream s11)
        # overlaps the exp phase.
        specs_a = [  # (dst tiles, dram src, ncols, nchunks, ss-row, dma engines)
            (Z[1], ht[1].ap(), N, NCH, 0, (nc.sync, nc.scalar)),
            (Zb[0], hb[0].ap(), R, NCHB, NCH, (nc.sync, nc.sync)),
            (Zb[1], hb[1].ap(), R, NCHB, NCH + NCHB, (nc.sync, nc.sync)),
        ]
        specs_b = [(Z[0], ht[0].ap(), N, NCH, 0, (nc.scalar, nc.sync))]

        with tc.tile_pool(name="work", bufs=5) as work, \
             tc.tile_pool(name="bbp", bufs=1) as bbp, \
             tc.tile_pool(name="prep_ps", bufs=3, space="PSUM") as pps:
            # all input loads issued up front, spread over both HW queues
            for specs in (specs_a, specs_b):
                for dst, src_, ncols, nch, srow, engs in specs:
                    for k in range(NK):
                        engs[k].dma_start(dst[k][:], src_[bass.ts(k, 128), :])

            def norm_pass(specs, ssk, rvk):
                for dst, src_, ncols, nch, srow, engs in specs:
                    for c in range(nch):
                        w = min(CH, ncols - c * CH)
                        cs = slice(c * CH, c * CH + w)
                        pn = pps.tile([1, CH], F32, tag="pnorm", name="pnorm")
                        for k in range(NK):
                            sq = work.tile([128, CH], BF16, tag="sq", name="sq")
                            nc.vector.tensor_mul(sq[:, :w], dst[k][:, cs],
                                                 dst[k][:, cs])
                            nc.tensor.matmul(pn[:, :w], ones_k[:], sq[:, :w],
                                             start=(k == 0), stop=(k == NK - 1))
                        stg = work.tile([1, CH], F32, tag="stg", name="stg")
                        nc.vector.tensor_copy(stg[:, :w], pn[:, :w])
                        r = srow + c
                        nc.gpsimd.dma_start(ssk[r:r + 1, :w], stg[:, :w])
                nrows = ssk.shape[0]
                nc.vector.reciprocal(ssk[:], ssk[:])
                nc.scalar.activation(ssk[:], ssk[:], AF.Sqrt, bias=zb[:nrows, :])
                nc.vector.tensor_copy(rvk[:], ssk[:])

            def scale_pass(specs, rvk, copies_on_act):
                # per 2048-group: broadcast + scale, so the first stream
                # groups can start before the whole tensor is scaled
                for dst, src_, ncols, nch, srow, engs in specs:
                    bb = bbp.tile([128, ncols], BF16, tag=f"bb{ncols}",
                                  name="bb")
                    for g0 in range(0, ncols, 2048):
                        ge = min(g0 + 2048, ncols)
                        for c in range(g0 // CH, (ge + CH - 1) // CH):
                            w = min(CH, ncols - c * CH)
                            cs = slice(c * CH, c * CH + w)
                            r = srow + c
                            rst = work.tile([1, CH], BF16, tag="rst",
                                            name="rst")
                            nc.gpsimd.dma_start(rst[:, :w], rvk[r:r + 1, :w])
                            pb = pps.tile([128, CH], F32, tag="pbcast",
                                          name="pbcast")
                            nc.tensor.matmul(pb[:, :w], ones_b[:],
                                             rst[0:1, :w],
                                             start=True, stop=True)
                            if copies_on_act:
                                nc.scalar.activation(bb[:, cs], pb[:, :w],
                                                     AF.Copy)
                            else:
                                nc.vector.tensor_copy(bb[:, cs], pb[:, :w])
                        for k in range(NK):
                            nc.vector.tensor_mul(dst[k][:, g0:ge],
                                                 dst[k][:, g0:ge],
                                                 bb[:, g0:ge])

            norm_pass(specs_a, sska, rvka)
            scale_pass(specs_a[1:] + specs_a[:1], rvka, copies_on_act=True)

            # pass B: Z0 chain, overlaps the s12/s22 exp streams
            norm_pass(specs_b, sskb, rvkb)
            scale_pass(specs_b, rvkb, copies_on_act=False)

            # --- S12 diagonal: sum_i z1_i . z2_i over own rows ---------
            prods = []
            for k in range(NK):
                pr = work.tile([128, R], BF16, tag=f"prod{k}", name=f"prod{k}")
                nc.vector.tensor_mul(pr[:], Zb[0][k][:], Zb[1][k][:])
                prods.append(pr)
            for c in range(NCHB):
                w = min(CHB, R - c * CHB)
                pii = pps.tile([1, CH], F32, tag="pnorm", name="pnorm")
                for k in range(NK):
                    nc.tensor.matmul(pii[:, :w], ones_k[:],
                                     prods[k][:, c * CHB:c * CHB + w],
                                     start=(k == 0), stop=(k == NK - 1))
                red = work.tile([1, 1], F32, tag="iired", name="iired")
                nc.vector.tensor_reduce(red[:], pii[:, :w], AX.X, ALU.add)
                if c == 0:
                    nc.vector.tensor_copy(ii_tot[:], red[:])
                else:
                    nc.vector.tensor_add(ii_tot[:], ii_tot[:], red[:])

        # --- exp/row-sum streams (S21 row-sums = S12 col-sums) --------
        streams = [("s12", Zb[0], Z[1]), ("s22", Zb[1], Z[1]),
                   ("s11", Zb[0], Z[0])]
        with (
            tc.tile_pool(name="spool", bufs=2, space="PSUM") as spool,
            tc.tile_pool(name="acc", bufs=4) as accp,
            tc.tile_pool(name="escp", bufs=4) as escp,
        ):
            for st, LHS, RHS in streams:
                for rt in range(NRT):
                    acc = accp.tile([128, NG], F32, tag="acc", name="acc")
                    for g in range(NG):
                        sg = spool.tile([128, G], F32, tag="sg", name="sg")
                        for k in range(NK):
                            for gc in range(GCH):
                                o = slice(gc * CH, (gc + 1) * CH)
                                col = slice(g * G + gc * CH,
                                            g * G + (gc + 1) * CH)
                                nc.tensor.matmul(sg[:, o],
                                                 LHS[k][:, bass.ts(rt, 128)],
                                                 RHS[k][:, col],
                                                 start=(k == 0),
                                                 stop=(k == NK - 1))
                        esc = escp.tile([128, G], BF16, tag="esc", name="esc")
                        if st == "s12":
                            # ACT-side row-sum: VectorE is saturated by the
                            # column accumulation during this stream
                            nc.scalar.activation(esc[:], sg[:], AF.Exp,
                                                 bias=zb[:], scale=1.0 / TAU,
                                                 accum_out=acc[:, g:g + 1])
                            gs = slice(g * G, (g + 1) * G)
                            if rt == 0:
                                nc.vector.tensor_copy(colacc[:, gs], esc[:])
                            else:
                                nc.vector.tensor_add(colacc[:, gs],
                                                     colacc[:, gs], esc[:])
                        else:
                            # s22/s11: VectorE is idle, so row-sum there and
                            # skip ACT's serial READ_ACCUMULATOR drain
                            nc.scalar.activation(esc[:], sg[:], AF.Exp,
                                                 bias=zb[:], scale=1.0 / TAU)
                            nc.vector.tensor_reduce(acc[:, g:g + 1], esc[:],
                                                    AX.X, ALU.add)
                    nc.vector.tensor_reduce(rows[st][:, rt:rt + 1], acc[:],
                                            AX.X, ALU.add)
                if st == "s12":
                    # S12 col-sums: reduce own 1024 rows, then sum row-blocks
                    # across cores; ReduceScatter hands each core its rows.
                    nc.gpsimd.partition_all_reduce(colacc[:], colacc[:], 128,
                                                   bass_isa.ReduceOp.add)
                    nc.sync.dma_start(cc_in[:], colacc[0:1, :])
                    nc.gpsimd.collective_compute(
                        "ReduceScatter", ALU.add,
                        replica_groups=[list(range(n_cores))],
                        ins=[cc_in.opt()], outs=[cc_out.opt()])
                    nc.sync.dma_start(dn[:],
                                      cc_out.rearrange("(t p) -> p t", p=128))

        # --- final: ln(den1*den2) summed, minus (2/tau)*sum(diag) -----
        with (
            tc.tile_pool(name="fin", bufs=1) as fin,
            tc.tile_pool(name="fin_ps", bufs=1, space="PSUM") as fps,
        ):
            den1 = fin.tile([128, NRT], F32, tag="den1", name="den1")
            den2 = fin.tile([128, NRT], F32, tag="den2", name="den2")
            nc.vector.tensor_add(den1[:], rows["s11"][:], rows["s12"][:])
            nc.vector.tensor_scalar_add(den1[:], den1[:], -E2)
            nc.vector.tensor_add(den2[:], rows["s22"][:], dn[:])
            nc.vector.tensor_scalar_add(den2[:], den2[:], -E2)
            dd = fin.tile([128, NRT], F32, tag="dd", name="dd")
            nc.vector.tensor_mul(dd[:], den1[:], den2[:])
            lnout = fin.tile([128, NRT], F32, tag="lnout", name="lnout")
            nc.scalar.activation(lnout[:], dd[:], AF.Ln, bias=zb[:],
                                 accum_out=lnacc[:])
            iim = fin.tile([1, 1], F32, tag="iim", name="iim")
            nc.vector.tensor_scalar_mul(iim[:], ii_tot[:], -2.0 / TAU)
            nc.vector.tensor_add(lnacc[0:1, :], lnacc[0:1, :], iim[:])
            ptot = fps.tile([1, 1], F32, tag="ptot", name="ptot")
            nc.tensor.matmul(ptot[:], ones_f[:], lnacc[:], start=True, stop=True)
            res = fin.tile([1, 1], F32, tag="res", name="res")
            nc.vector.tensor_copy(res[:], ptot[:])
            nc.sync.dma_start(out.ap()[:, :], res[:])

    nc.compile()
    return nc


_CACHE = {}


def _compiled(N=N_FULL, D=D_FULL, n_cores=N_CORES):
    key = (N, D, n_cores)
    if key not in _CACHE:
        _CACHE[key] = build_nc(N, D, n_cores)
    return _CACHE[key]


def make_in_maps(H_1, H_2, n_cores=N_CORES):
    H1 = np.asarray(H_1, dtype=np.float32)
    H2 = np.asarray(H_2, dtype=np.float32)
    N = H1.shape[0]
    R = N // n_cores
    HT1 = np.ascontiguousarray(H1.astype(ml_dtypes.bfloat16).T)
    HT2 = np.ascontiguousarray(H2.astype(ml_dtypes.bfloat16).T)
    maps = []
    for c in range(n_cores):
        sl = slice(c * R, (c + 1) * R)
        maps.append({
            "HT1": HT1, "HT2": HT2,
            "Hb1": np.ascontiguousarray(HT1[:, sl]),
            "Hb2": np.ascontiguousarray(HT2[:, sl]),
        })
    return maps


def kernel(H_1, H_2):
    N, D = H_1.shape
    nc = _compiled(N, D, N_CORES)
    in_maps = make_in_maps(H_1, H_2, N_CORES)
    res = run_bass_kernel_spmd(nc, in_maps, core_ids=list(range(N_CORES)))
    total = sum(float(r["out"][0, 0]) for r in res.results)
    return np.float32(total / (2.0 * N))



# revision 5
# speedup vs baseline: 1.0515x; 1.0515x over previous
"""Trainium2 8-core kernel for the paired contrastive (NT-Xent-like) loss.

Math (tau=0.5, N=8192, D=256):
    z1 = l2norm(H_1), z2 = l2norm(H_2)
    den1_i = sum_j exp(z1.z1/t) + sum_j exp(z1.z2/t) - exp(1/t)
    den2_i = sum_j exp(z2.z2/t) + sum_j exp(z2.z1/t) - exp(1/t)
    loss = (1/2N) * sum_i [ ln(den1_i) + ln(den2_i) - 2*(z1_i.z2_i)/t ]

S11 and S22 are symmetric, so only their upper triangles are computed
(2N^2 exps instead of 3N^2).  Work is balanced by pairing 128-row tiles:
row-tile r in [0,32) computes cyclic column distances 0..32, r in [32,64)
distances 0..31 -- every unordered tile pair is covered exactly once.

Each core owns 8 row-tiles {4c+u} u {32+4c+u} (u<4).  To keep the SPMD
graph identical across cores, the host hands every core its embeddings
with columns ROTATED by 4c tiles (plus 3 duplicated tiles appended), so
each core's stationary blocks and triangle windows sit at the same local
offsets.  Row sums come from the ACT accumulator (fused with exp); column
sums are accumulated in bf16 SBUF tensors (DVE/Pool adds) and reduced
across partitions with a ones-vector matmul.  The host un-rotates the
partial sums, assembles den1/den2, and takes the final log/mean (a few
hundred KB of O(N) work).
"""

import math

import numpy as np
import ml_dtypes

import concourse.bass as bass
import concourse.tile as tile
from concourse import bacc, mybir
from concourse.bass_utils import run_bass_kernel_spmd

F32 = mybir.dt.float32
BF16 = mybir.dt.bfloat16
AF = mybir.ActivationFunctionType
ALU = mybir.AluOpType
AX = mybir.AxisListType

TAU = 0.5
E2 = math.exp(1.0 / TAU)

N_FULL, D_FULL, N_CORES = 8192, 256, 8
TI = 128
T = N_FULL // TI            # 64 column tiles
EXTT = T + 3                # 3 duplicated tiles so every window is contiguous
EXTC = EXTT * TI            # 8576
NK = D_FULL // TI           # 2 contraction k-tiles
CH = 512                    # matmul moving chunk (one PSUM bank)
G = 2048                    # exp group (4 PSUM banks)

# stationary slots: local tile positions 0..3 and 32..35
SPOS = [TI * u for u in range(4)] + [4096 + TI * u for u in range(4)]


def _windows(st, s):
    """(start, width) of the moving-column window for stream st, slot s."""
    if st == "s12":
        return 0, N_FULL
    u = s % 4
    if s < 4:
        return TI * u, 33 * TI      # distances 0..32
    return 4096 + TI * u, 32 * TI   # distances 0..31


def _groups(w0, ww):
    out = []
    o = 0
    while o < ww:
        gw = min(G, ww - o)
        out.append((w0 + o, gw))
        o += gw
    return out


def build_nc(n_cores=N_CORES):
    nc = bacc.Bacc("TRN2", target_bir_lowering=False, debug=False,
                   num_devices=n_cores)

    m_in = [nc.dram_tensor("M1", [D_FULL, EXTC], BF16, kind="ExternalInput"),
            nc.dram_tensor("M2", [D_FULL, EXTC], BF16, kind="ExternalInput")]
    cs_out = {"s12": nc.dram_tensor("cs12", [N_FULL], F32, kind="ExternalOutput"),
              "s22": nc.dram_tensor("cs22", [EXTC], F32, kind="ExternalOutput"),
              "s11": nc.dram_tensor("cs11", [EXTC], F32, kind="ExternalOutput")}
    rows_out = {st: nc.dram_tensor(f"r{st[1:]}", [TI, 8], F32,
                                   kind="ExternalOutput")
                for st in ("s11", "s12", "s22")}
    ii_out = nc.dram_tensor("ii", [1, 1], F32, kind="ExternalOutput")

    NCH = N_FULL // CH          # 16 norm chunks per tensor

    with tile.TileContext(nc) as tc, \
         tc.tile_pool(name="persist", bufs=1) as per:
        Z = [[per.tile([TI, EXTC], BF16, tag=f"z{t}{k}", name=f"z{t}{k}")
              for k in range(NK)] for t in range(2)]
        CA = {"s12": per.tile([TI, N_FULL], BF16, tag="ca12", name="ca12"),
              "s22": per.tile([TI, EXTC], BF16, tag="ca22", name="ca22"),
              "s11": per.tile([TI, EXTC], BF16, tag="ca11", name="ca11")}
        rows = {st: per.tile([TI, 8], F32, tag=f"rows_{st}", name=f"rows_{st}")
                for st in ("s11", "s12", "s22")}
        sska = per.tile([2 * NCH, CH], F32, tag="sska", name="sska")
        rvka = per.tile([2 * NCH, CH], BF16, tag="rvka", name="rvka")
        ii_tot = per.tile([1, 1], F32, tag="ii_tot", name="ii_tot")
        ones_k = per.tile([TI, 1], BF16, tag="ones_k", name="ones_k")
        ones_b = per.tile([1, TI], BF16, tag="ones_b", name="ones_b")
        zb = per.tile([TI, 1], F32, tag="zb", name="zb")

        nc.gpsimd.memset(ones_k[:], 1.0)
        nc.gpsimd.memset(ones_b[:], 1.0)
        nc.gpsimd.memset(zb[:], 0.0)
        for st in CA:
            nc.vector.memset(CA[st][:], 0.0)

        # ---- input DMA, split into col-blocks across both HW queues ----
        DB = EXTC // 4
        engs = (nc.sync, nc.scalar)
        for blk in range(4):
            cs = slice(blk * DB, (blk + 1) * DB)
            for t in range(2):
                for k in range(NK):
                    engs[(t + k) % 2].dma_start(Z[t][k][:, cs],
                                                m_in[t].ap()[bass.ts(k, TI), cs])

        # ---- norms + scale ------------------------------------------------
        with tc.tile_pool(name="work", bufs=5) as work, \
             tc.tile_pool(name="pps", bufs=2, space="PSUM") as pps:
            for t in range(2):
                for c in range(NCH):
                    cs = slice(c * CH, (c + 1) * CH)
                    pn = pps.tile([1, CH], F32, tag="pn", name="pn")
                    for k in range(NK):
                        sq = work.tile([TI, CH], BF16, tag="sq", name="sq")
                        nc.vector.tensor_mul(sq[:], Z[t][k][:, cs], Z[t][k][:, cs])
                        nc.tensor.matmul(pn[:], ones_k[:], sq[:],
                                         start=(k == 0), stop=(k == NK - 1))
                    stg = work.tile([1, CH], F32, tag="stg", name="stg")
                    nc.vector.tensor_copy(stg[:], pn[:])
                    r = t * NCH + c
                    nc.gpsimd.dma_start(sska[r:r + 1, :], stg[:])
            nc.vector.reciprocal(sska[:], sska[:])
            nc.scalar.activation(sska[:], sska[:], AF.Sqrt, bias=zb[:2 * NCH, :])
            nc.vector.tensor_copy(rvka[:], sska[:])

            # scale: 16 full chunks + the 384-wide dup tail per tensor
            for t in range(2):
                for c in range(NCH + 1):
                    if c < NCH:
                        w, c0, r, ro = CH, c * CH, t * NCH + c, 0
                    else:
                        w, c0, r, ro = EXTC - N_FULL, N_FULL, t * NCH, 0
                    cs = slice(c0, c0 + w)
                    rst = work.tile([1, CH], BF16, tag="rst", name="rst")
                    nc.gpsimd.dma_start(rst[:, :w], rvka[r:r + 1, ro:ro + w])
                    pb = pps.tile([TI, CH], F32, tag="pb", name="pb")
                    nc.tensor.matmul(pb[:, :w], ones_b[:], rst[0:1, :w],
                                     start=True, stop=True)
                    bb = work.tile([TI, CH], BF16, tag="bb", name="bb")
                    nc.scalar.activation(bb[:, :w], pb[:, :w], AF.Copy)
                    for k in range(NK):
                        nc.vector.tensor_mul(Z[t][k][:, cs], Z[t][k][:, cs],
                                             bb[:, :w])

            # ---- ii = sum over own rows of z1.z2 -------------------------
            for s in range(8):
                so = SPOS[s]
                ss = slice(so, so + TI)
                pii = pps.tile([1, TI], F32, tag="pii", name="pii")
                for k in range(NK):
                    pr = work.tile([TI, TI], BF16, tag="pr", name="pr")
                    nc.vector.tensor_mul(pr[:], Z[0][k][:, ss], Z[1][k][:, ss])
                    nc.tensor.matmul(pii[:], ones_k[:], pr[:],
                                     start=(k == 0), stop=(k == NK - 1))
                red = work.tile([1, 1], F32, tag="red", name="red")
                nc.vector.tensor_reduce(red[:], pii[:], AX.X, ALU.add)
                if s == 0:
                    nc.vector.tensor_copy(ii_tot[:], red[:])
                else:
                    nc.vector.tensor_add(ii_tot[:], ii_tot[:], red[:])
            nc.sync.dma_start(ii_out.ap()[:, :], ii_tot[:])

        # ---- exp streams --------------------------------------------------
        streams = [("s12", Z[0], Z[1]), ("s22", Z[1], Z[1]),
                   ("s11", Z[0], Z[0])]
        with (
            tc.tile_pool(name="spool", bufs=2, space="PSUM") as spool,
            tc.tile_pool(name="accp", bufs=2) as accp,
            tc.tile_pool(name="escp", bufs=4) as escp,
        ):
            for st, ZS, ZM in streams:
                ca = CA[st]
                # s12 colacc adds on DVE; s22 on Pool; s11 split
                for s in range(8):
                    so = SPOS[s]
                    w0, ww = _windows(st, s)
                    grps = _groups(w0, ww)
                    acc = accp.tile([TI, 4], F32, tag="acc", name="acc")
                    for gi, (g0, gw) in enumerate(grps):
                        sg = spool.tile([TI, G], F32, tag="sg", name="sg")
                        for k in range(NK):
                            for o in range(0, gw, CH):
                                cw = min(CH, gw - o)
                                nc.tensor.matmul(
                                    sg[:, o:o + cw],
                                    ZS[k][:, so:so + TI],
                                    ZM[k][:, g0 + o:g0 + o + cw],
                                    start=(k == 0), stop=(k == NK - 1))
                        esc = escp.tile([TI, G], BF16, tag="esc", name="esc")
                        nc.scalar.activation(esc[:, :gw], sg[:, :gw], AF.Exp,
                                             bias=zb[:], scale=1.0 / TAU,
                                             accum_out=acc[:, gi:gi + 1])
                        # column-sum accumulate; skip the diagonal tile for
                        # the symmetric streams (rows already cover it)
                        do = TI if (st != "s12" and gi == 0) else 0
                        if gw - do > 0:
                            eng = nc.vector if st == "s12" or (st == "s11" and s % 2 == 0) \
                                else nc.gpsimd
                            eng.tensor_add(ca[:, g0 + do:g0 + gw],
                                           ca[:, g0 + do:g0 + gw],
                                           esc[:, do:gw])
                    nc.vector.tensor_reduce(rows[st][:, s:s + 1],
                                            acc[:, :len(grps)], AX.X, ALU.add)
                nc.sync.dma_start(rows_out[st].ap()[:, :], rows[st][:])

        # ---- column-sum partition reduce (ones-matmul) + DMA out ---------
        with tc.tile_pool(name="fps", bufs=2, space="PSUM") as fps, \
             tc.tile_pool(name="fsb", bufs=4) as fsb:
            pi = 0
            for st in ("s12", "s22", "s11"):
                wtot = CA[st].shape[1]
                for o in range(0, wtot, G):
                    w = min(G, wtot - o)
                    pc = fps.tile([1, G], F32, tag="pc", name="pc")
                    for j in range(0, w, CH):
                        jw = min(CH, w - j)
                        nc.tensor.matmul(pc[:, j:j + jw], ones_k[:],
                                         CA[st][:, o + j:o + j + jw],
                                         start=True, stop=True)
                    sc = fsb.tile([1, G], F32, tag="sc", name="sc")
                    if pi % 2 == 0:
                        nc.vector.tensor_copy(sc[:, :w], pc[:, :w])
                    else:
                        nc.scalar.activation(sc[:, :w], pc[:, :w], AF.Copy)
                    nc.sync.dma_start(cs_out[st].ap()[o:o + w], sc[0:1, :w])
                    pi += 1

    nc.compile()
    return nc


_CACHE = {}


def _compiled(n_cores=N_CORES):
    if n_cores not in _CACHE:
        _CACHE[n_cores] = build_nc(n_cores)
    return _CACHE[n_cores]


def _perm(c):
    p = np.arange(EXTC)
    return TI * ((4 * c + p // TI) % T) + p % TI


def make_in_maps(H_1, H_2, n_cores=N_CORES):
    HT1 = np.ascontiguousarray(
        np.asarray(H_1, np.float32).astype(ml_dtypes.bfloat16).T)
    HT2 = np.ascontiguousarray(
        np.asarray(H_2, np.float32).astype(ml_dtypes.bfloat16).T)
    maps = []
    for c in range(n_cores):
        pm = _perm(c)
        maps.append({"M1": np.ascontiguousarray(HT1[:, pm]),
                     "M2": np.ascontiguousarray(HT2[:, pm])})
    return maps


def finalize(results, n_cores=N_CORES):
    N = N_FULL
    den1 = np.zeros(N, np.float64)
    den2 = np.zeros(N, np.float64)
    ii_sum = 0.0
    for c in range(n_cores):
        r = results[c]
        pm = _perm(c)
        rowtiles = [4 * c + u for u in range(4)] + \
                   [32 + 4 * c + u for u in range(4)]
        r11 = np.asarray(r["r11"], np.float64)
        r12 = np.asarray(r["r12"], np.float64)
        r22 = np.asarray(r["r22"], np.float64)
        for s, rt in enumerate(rowtiles):
            gr = slice(TI * rt, TI * (rt + 1))
            den1[gr] += r12[:, s] + r11[:, s]
            den2[gr] += r22[:, s]
        np.add.at(den2, pm[:N], np.asarray(r["cs12"], np.float64))
        np.add.at(den1, pm, np.asarray(r["cs11"], np.float64))
        np.add.at(den2, pm, np.asarray(r["cs22"], np.float64))
        ii_sum += float(np.asarray(r["ii"])[0, 0])
    den1 -= E2
    den2 -= E2
    loss = (np.sum(np.log(den1)) + np.sum(np.log(den2))
            - (2.0 / TAU) * ii_sum) / (2.0 * N)
    return np.float32(loss)


def kernel(H_1, H_2):
    nc = _compiled(N_CORES)
    in_maps = make_in_maps(H_1, H_2, N_CORES)
    res = run_bass_kernel_spmd(nc, in_maps, core_ids=list(range(N_CORES)))
    return finalize(res.results, N_CORES)


# revision 12
# speedup vs baseline: 1.1831x; 1.1252x over previous
"""Trainium2 8-core kernel for the paired contrastive (NT-Xent-like) loss.

Math (tau=0.5, N=8192, D=256):
    z1 = l2norm(H_1), z2 = l2norm(H_2)
    den1_i = sum_j exp(z1.z1/t) + sum_j exp(z1.z2/t) - exp(1/t)
    den2_i = sum_j exp(z2.z2/t) + sum_j exp(z2.z1/t) - exp(1/t)
    loss = (1/2N) * sum_i [ ln(den1_i) + ln(den2_i) - 2*(z1_i.z2_i)/t ]

S11 and S22 are symmetric, so only their upper triangles are computed
(2N^2 exps instead of 3N^2).  Work is balanced by pairing 128-row tiles:
row-tile r in [0,32) computes cyclic column distances 0..32, r in [32,64)
distances 0..31 -- every unordered tile pair is covered exactly once.

Each core owns 8 row-tiles {4c+u} u {32+4c+u} (u<4).  To keep the SPMD
graph identical across cores, the host hands every core its embeddings
with columns ROTATED by 4c tiles (plus 3 duplicated tiles appended), so
each core's stationary blocks and triangle windows sit at the same local
offsets.  Row sums come from the ACT accumulator (fused with exp); column
sums are accumulated in bf16 SBUF tensors (DVE/Pool adds) and reduced
across partitions with a ones-vector matmul.  The host un-rotates the
partial sums, assembles den1/den2, and takes the final log/mean (a few
hundred KB of O(N) work).
"""

import math

import numpy as np
import ml_dtypes

import concourse.bass as bass
import concourse.tile as tile
from concourse import bacc, mybir
from concourse.bass_utils import run_bass_kernel_spmd

F32 = mybir.dt.float32
BF16 = mybir.dt.bfloat16
AF = mybir.ActivationFunctionType
ALU = mybir.AluOpType
AX = mybir.AxisListType

TAU = 0.5
E2 = math.exp(1.0 / TAU)

N_FULL, D_FULL, N_CORES = 8192, 256, 8
TI = 128
T = N_FULL // TI            # 64 column tiles
EXTT = T + 3                # 3 duplicated tiles so every window is contiguous
EXTC = EXTT * TI            # 8576
NK = D_FULL // TI           # 2 contraction k-tiles
CH = 512                    # matmul moving chunk (one PSUM bank)
G = 2048                    # exp group (4 PSUM banks)

# stationary slots: local tile positions 0..3 and 32..35
SPOS = [TI * u for u in range(4)] + [4096 + TI * u for u in range(4)]


def _windows(st, s):
    """(start, width) of the moving-column window for stream st, slot s."""
    if st == "s12":
        return 0, N_FULL
    u = s % 4
    if s < 4:
        return TI * u, 33 * TI      # distances 0..32
    return 4096 + TI * u, 32 * TI   # distances 0..31


def _groups(w0, ww):
    out = []
    o = 0
    while o < ww:
        gw = min(G, ww - o)
        out.append((w0 + o, gw))
        o += gw
    return out


def build_nc(n_cores=N_CORES):
    nc = bacc.Bacc("TRN2", target_bir_lowering=False, debug=False,
                   num_devices=n_cores)

    m_in = [nc.dram_tensor("M1", [D_FULL, EXTC], BF16, kind="ExternalInput"),
            nc.dram_tensor("M2", [D_FULL, EXTC], BF16, kind="ExternalInput")]
    cs_out = {"s12": nc.dram_tensor("cs12", [N_FULL], F32, kind="ExternalOutput"),
              "s22": nc.dram_tensor("cs22", [EXTC], F32, kind="ExternalOutput"),
              "s11": nc.dram_tensor("cs11", [EXTC], F32, kind="ExternalOutput")}
    rows_out = {st: nc.dram_tensor(f"r{st[1:]}", [TI, 8], F32,
                                   kind="ExternalOutput")
                for st in ("s11", "s12", "s22")}
    ii_out = nc.dram_tensor("ii", [1, 1], F32, kind="ExternalOutput")

    NCH = N_FULL // CH          # 16 norm chunks per tensor

    with tile.TileContext(nc) as tc, \
         tc.tile_pool(name="persist", bufs=1) as per:
        Z = [[per.tile([TI, EXTC], BF16, tag=f"z{t}{k}", name=f"z{t}{k}")
              for k in range(NK)] for t in range(2)]
        CA = {"s12": per.tile([TI, N_FULL], BF16, tag="ca12", name="ca12"),
              "s22": per.tile([TI, EXTC], BF16, tag="ca22", name="ca22"),
              "s11": per.tile([TI, EXTC], BF16, tag="ca11", name="ca11")}
        rows = {st: per.tile([TI, 8], F32, tag=f"rows_{st}", name=f"rows_{st}")
                for st in ("s11", "s12", "s22")}
        sska = [per.tile([NCH, CH], F32, tag=f"sska{t}", name=f"sska{t}")
                for t in range(2)]
        rvka = [per.tile([NCH, CH], BF16, tag=f"rvka{t}", name=f"rvka{t}")
                for t in range(2)]
        ii_tot = per.tile([1, 1], F32, tag="ii_tot", name="ii_tot")
        ones_k = per.tile([TI, 1], BF16, tag="ones_k", name="ones_k")
        ones_b = per.tile([1, TI], BF16, tag="ones_b", name="ones_b")
        zb = per.tile([TI, 1], F32, tag="zb", name="zb")

        nc.gpsimd.memset(ones_k[:], 1.0)
        nc.gpsimd.memset(ones_b[:], 1.0)
        nc.gpsimd.memset(zb[:], 0.0)
        # ca12 is initialized by a first-touch copy; split the others
        # between DVE and Pool so neither blocks the ramp
        half = EXTC // 2
        nc.vector.memset(CA["s22"][:, :half], 0.0)
        nc.gpsimd.memset(CA["s22"][:, half:], 0.0)
        nc.vector.memset(CA["s11"][:, :half], 0.0)
        nc.gpsimd.memset(CA["s11"][:, half:], 0.0)

        # ---- input DMA, split into col-blocks across both HW queues ----
        blocks = [(0, 2048), (2048, 2048), (4096, 2048), (6144, EXTC - 6144)]
        engs = (nc.sync, nc.scalar)
        for b0, bw in blocks:
            cs = slice(b0, b0 + bw)
            for t in range(2):
                for k in range(NK):
                    engs[(t + k) % 2].dma_start(Z[t][k][:, cs],
                                                m_in[t].ap()[bass.ts(k, TI), cs])

        # ---- norms + scale ------------------------------------------------
        with tc.tile_pool(name="work", bufs=5) as work, \
             tc.tile_pool(name="pps", bufs=2, space="PSUM") as pps:

            def norm_chain(t, scale_order):
                for c in range(NCH):
                    cs = slice(c * CH, (c + 1) * CH)
                    pn = pps.tile([1, CH], F32, tag="pn", name="pn")
                    for k in range(NK):
                        sq = work.tile([TI, CH], BF16, tag="sq", name="sq")
                        nc.vector.tensor_mul(sq[:], Z[t][k][:, cs], Z[t][k][:, cs])
                        nc.tensor.matmul(pn[:], ones_k[:], sq[:],
                                         start=(k == 0), stop=(k == NK - 1))
                    stg = work.tile([1, CH], F32, tag="stg", name="stg")
                    nc.vector.tensor_copy(stg[:], pn[:])
                    nc.gpsimd.dma_start(sska[t][c:c + 1, :], stg[:])
                nc.vector.reciprocal(sska[t][:], sska[t][:])
                nc.scalar.activation(sska[t][:], sska[t][:], AF.Sqrt,
                                     bias=zb[:NCH, :])
                nc.vector.tensor_copy(rvka[t][:], sska[t][:])
                for c in scale_order:
                    if c < NCH:
                        w, c0, r = CH, c * CH, c
                    else:
                        w, c0, r = EXTC - N_FULL, N_FULL, 0
                    cs = slice(c0, c0 + w)
                    rst = work.tile([1, CH], BF16, tag="rst", name="rst")
                    nc.gpsimd.dma_start(rst[:, :w], rvka[t][r:r + 1, :w])
                    pb = pps.tile([TI, CH], F32, tag="pb", name="pb")
                    nc.tensor.matmul(pb[:, :w], ones_b[:], rst[0:1, :w],
                                     start=True, stop=True)
                    bb = work.tile([TI, CH], BF16, tag="bb", name="bb")
                    nc.scalar.activation(bb[:, :w], pb[:, :w], AF.Copy)
                    for k in range(NK):
                        nc.vector.tensor_mul(Z[t][k][:, cs], Z[t][k][:, cs],
                                             bb[:, :w])

            # M2 first (s12/s22 moving operand), then M1 with its
            # stationary chunks (0 and 8) scaled first
            norm_chain(1, list(range(NCH + 1)))
            norm_chain(0, [0, 8] + [c for c in range(NCH + 1)
                                    if c not in (0, 8)])

            # ---- ii = sum over own rows of z1.z2 -------------------------
            for s in range(8):
                so = SPOS[s]
                ss = slice(so, so + TI)
                pii = pps.tile([1, TI], F32, tag="pii", name="pii")
                for k in range(NK):
                    pr = work.tile([TI, TI], BF16, tag="pr", name="pr")
                    nc.vector.tensor_mul(pr[:], Z[0][k][:, ss], Z[1][k][:, ss])
                    nc.tensor.matmul(pii[:], ones_k[:], pr[:],
                                     start=(k == 0), stop=(k == NK - 1))
                red = work.tile([1, 1], F32, tag="red", name="red")
                nc.vector.tensor_reduce(red[:], pii[:], AX.X, ALU.add)
                if s == 0:
                    nc.vector.tensor_copy(ii_tot[:], red[:])
                else:
                    nc.vector.tensor_add(ii_tot[:], ii_tot[:], red[:])
            nc.sync.dma_start(ii_out.ap()[:, :], ii_tot[:])

        # ---- exp streams --------------------------------------------------
        streams = [("s12", Z[0], Z[1]), ("s22", Z[1], Z[1]),
                   ("s11", Z[0], Z[0])]
        with (
            tc.tile_pool(name="spool", bufs=2, space="PSUM") as spool,
            tc.tile_pool(name="accp", bufs=2) as accp,
            tc.tile_pool(name="escp", bufs=4) as escp,
        ):
            for st, ZS, ZM in streams:
                ca = CA[st]
                # s12 colacc adds on DVE; s22 on Pool; s11 split
                for s in range(8):
                    so = SPOS[s]
                    w0, ww = _windows(st, s)
                    grps = _groups(w0, ww)
                    acc = accp.tile([TI, 4], F32, tag="acc", name="acc")
                    for gi, (g0, gw) in enumerate(grps):
                        sg = spool.tile([TI, G], F32, tag="sg", name="sg")
                        for k in range(NK):
                            for o in range(0, gw, CH):
                                cw = min(CH, gw - o)
                                nc.tensor.matmul(
                                    sg[:, o:o + cw],
                                    ZS[k][:, so:so + TI],
                                    ZM[k][:, g0 + o:g0 + o + cw],
                                    start=(k == 0), stop=(k == NK - 1))
                        esc = escp.tile([TI, G], BF16, tag="esc", name="esc")
                        nc.scalar.activation(esc[:, :gw], sg[:, :gw], AF.Exp,
                                             bias=zb[:], scale=1.0 / TAU,
                                             accum_out=acc[:, gi:gi + 1])
                        # column-sum accumulate; skip the diagonal tile for
                        # the symmetric streams (rows already cover it)
                        do = TI if (st != "s12" and gi == 0) else 0
                        if gw - do > 0:
                            if st == "s12" and s == 0:
                                nc.vector.tensor_copy(ca[:, g0:g0 + gw],
                                                      esc[:, :gw])
                            else:
                                nc.vector.tensor_add(ca[:, g0 + do:g0 + gw],
                                                     ca[:, g0 + do:g0 + gw],
                                                     esc[:, do:gw])
                    nc.vector.tensor_reduce(rows[st][:, s:s + 1],
                                            acc[:, :len(grps)], AX.X, ALU.add)
                nc.sync.dma_start(rows_out[st].ap()[:, :], rows[st][:])

        # ---- column-sum partition reduce (ones-matmul) + DMA out ---------
        with tc.tile_pool(name="fps", bufs=2, space="PSUM") as fps, \
             tc.tile_pool(name="fsb", bufs=4) as fsb:
            pi = 0
            for st in ("s12", "s22", "s11"):
                wtot = CA[st].shape[1]
                for o in range(0, wtot, G):
                    w = min(G, wtot - o)
                    pc = fps.tile([1, G], F32, tag="pc", name="pc")
                    for j in range(0, w, CH):
                        jw = min(CH, w - j)
                        nc.tensor.matmul(pc[:, j:j + jw], ones_k[:],
                                         CA[st][:, o + j:o + j + jw],
                                         start=True, stop=True)
                    sc = fsb.tile([1, G], F32, tag="sc", name="sc")
                    if pi % 2 == 0:
                        nc.vector.tensor_copy(sc[:, :w], pc[:, :w])
                    else:
                        nc.scalar.activation(sc[:, :w], pc[:, :w], AF.Copy)
                    dq = (nc.sync, nc.scalar, nc.gpsimd)[pi % 3]
                    dq.dma_start(cs_out[st].ap()[o:o + w], sc[0:1, :w])
                    pi += 1

    nc.compile()
    return nc


_CACHE = {}


def _compiled(n_cores=N_CORES):
    if n_cores not in _CACHE:
        _CACHE[n_cores] = build_nc(n_cores)
    return _CACHE[n_cores]


def _perm(c):
    p = np.arange(EXTC)
    return TI * ((4 * c + p // TI) % T) + p % TI


def make_in_maps(H_1, H_2, n_cores=N_CORES):
    HT1 = np.ascontiguousarray(
        np.asarray(H_1, np.float32).astype(ml_dtypes.bfloat16).T)
    HT2 = np.ascontiguousarray(
        np.asarray(H_2, np.float32).astype(ml_dtypes.bfloat16).T)
    maps = []
    for c in range(n_cores):
        pm = _perm(c)
        maps.append({"M1": np.ascontiguousarray(HT1[:, pm]),
                     "M2": np.ascontiguousarray(HT2[:, pm])})
    return maps


def finalize(results, n_cores=N_CORES):
    N = N_FULL
    den1 = np.zeros(N, np.float64)
    den2 = np.zeros(N, np.float64)
    ii_sum = 0.0
    for c in range(n_cores):
        r = results[c]
        pm = _perm(c)
        rowtiles = [4 * c + u for u in range(4)] + \
                   [32 + 4 * c + u for u in range(4)]
        r11 = np.asarray(r["r11"], np.float64)
        r12 = np.asarray(r["r12"], np.float64)
        r22 = np.asarray(r["r22"], np.float64)
        for s, rt in enumerate(rowtiles):
            gr = slice(TI * rt, TI * (rt + 1))
            den1[gr] += r12[:, s] + r11[:, s]
            den2[gr] += r22[:, s]
        np.add.at(den2, pm[:N], np.asarray(r["cs12"], np.float64))
        np.add.at(den1, pm, np.asarray(r["cs11"], np.float64))
        np.add.at(den2, pm, np.asarray(r["cs22"], np.float64))
        ii_sum += float(np.asarray(r["ii"])[0, 0])
    den1 -= E2
    den2 -= E2
    loss = (np.sum(np.log(den1)) + np.sum(np.log(den2))
            - (2.0 / TAU) * ii_sum) / (2.0 * N)
    return np.float32(loss)


def kernel(H_1, H_2):
    nc = _compiled(N_CORES)
    in_maps = make_in_maps(H_1, H_2, N_CORES)
    res = run_bass_kernel_spmd(nc, in_maps, core_ids=list(range(N_CORES)))
    return finalize(res.results, N_CORES)
